# revision 4
# baseline (speedup 1.0000x reference)
"""Trainium2 Bass kernel for nn_ContextPredictionModel (dense_cnn).

Contract: kernel(**inputs) takes FULL unsharded inputs (numpy), returns the
FULL [120, 256, 1024] f32 output. Internally shards batch B=256 across 8
NeuronCores (data parallel) and syncs BatchNorm statistics with AllReduce.

Math notes (vs reference):
  - conv biases of layers 0 and 1 are channel-constant shifts of the next
    BatchNorm's input, so they cancel exactly in BN -> dropped.
  - layer-2 conv bias + the 1/9 avg-pool factor are folded on the host into
    the prediction-head weights/biases:
        pred = W @ (pool_sum/9 + b2) + lb = (W/9) @ pool_sum + (W @ b2 + lb)
  - BN per-(patch, channel) stats over (batch, 3x3): each core accumulates
    local (sum, sumsq), one AllReduce per layer boundary merges them.
"""

import os
import numpy as np
import ml_dtypes

import concourse.bass as bass
import concourse.mybir as mybir
import concourse.tile as tile
from concourse import bacc
from concourse import bass_utils

# ---------------- problem constants (hardcoded; self-contained) -------------
B_FULL = 256
C_FULL = 1024
HW = 7
NL = 3
NPATCH = 25
KPIX = 9  # 3x3
NCORES = 8
EPS = 1e-5
NHEADS = 12

# matmul/storage dtype: "bf16" | "f32r" | "f32" (env override for experiments)
DTYPE = os.environ.get("CPM_DTYPE", "bf16")
TRACE = False  # set True from test harness to capture NTFF profile
LAST_RESULT = None  # BassKernelResults of last kernel() call

_AF = mybir.ActivationFunctionType
_ALU = mybir.AluOpType


def _head_patch_ids(d):
    ids = []
    for y in range(5):
        for x in range(5):
            p = y * 5 + x
            if d == 0 and y in (0, 1):
                ids.append(p)
            elif d == 1 and y in (3, 4):
                ids.append(p)
            elif d == 2 and x in (0, 1):
                ids.append(p)
            elif d == 3 and x in (3, 4):
                ids.append(p)
    return ids


def _pred_index_map():
    """m[h, i] = row in the final [120, B, C] output for the i-th
    (ascending-p) patch of head h (h = d*3 + s)."""
    m = np.zeros((NHEADS, 10), dtype=np.int64)
    cnt = [0] * NHEADS
    j = 0
    for y1 in range(5):
        for x1 in range(5):
            conds = []
            if y1 + 2 in (2, 3):
                conds.append(0)
            if y1 in (3, 4):
                conds.append(1)
            if x1 + 2 in (2, 3):
                conds.append(2)
            if x1 in (3, 4):
                conds.append(3)
            for d in conds:
                for s in range(3):
                    h = d * 3 + s
                    m[h, cnt[h]] = j
                    cnt[h] += 1
                    j += 1
    assert j == 120 and all(c == 10 for c in cnt)
    return m


def _dt_pair(dt_str):
    """(mybir storage dtype for matmul operands, numpy dtype for host arrays)."""
    if dt_str == "bf16":
        return mybir.dt.bfloat16, ml_dtypes.bfloat16
    if dt_str == "f32r":
        return mybir.dt.float32r, np.float32
    if dt_str == "f32":
        return mybir.dt.float32, np.float32
    raise ValueError(dt_str)


def build_nc(ncores=NCORES, bl=B_FULL // NCORES, c=C_FULL, dt_str=DTYPE):
    """Build + compile the per-core Bass program (SPMD, same on all cores)."""
    D, _ = _dt_pair(dt_str)
    f32 = mybir.dt.float32
    nct = c // 128            # channel tiles
    nact = bl * KPIX          # conv matmul free dim per patch
    ntot = ncores * bl * KPIX  # global BN count per (patch, channel)
    nstat = nct * NPATCH * 2
    n_out_chunks = (c + 511) // 512
    # head M-tiles: groups of whole patches with <=128 rows
    ppt = max(1, min(10, 128 // bl))
    mt_groups = []
    p0 = 0
    while p0 < 10:
        npat = min(ppt, 10 - p0)
        mt_groups.append((p0, npat))
        p0 += npat

    nc = bacc.Bacc("TRN2", target_bir_lowering=False, debug=False,
                   num_devices=ncores)

    # ---------------- I/O ----------------
    x_in = nc.dram_tensor("x_t", [c, bl, HW * HW], f32, kind="ExternalInput")
    cw_in = nc.dram_tensor("cw_t", [NL, c, c], D, kind="ExternalInput")
    lw_in = nc.dram_tensor("lw_t", [NHEADS, c, c], D, kind="ExternalInput")
    lb_in = nc.dram_tensor("lb_t", [NHEADS, c], D, kind="ExternalInput")
    gam_in = nc.dram_tensor("gam_t", [NL, c], f32, kind="ExternalInput")
    bet_in = nc.dram_tensor("bet_t", [NL, c], f32, kind="ExternalInput")
    preds_out = nc.dram_tensor("preds_t", [NHEADS, 10, bl, c], f32,
                               kind="ExternalOutput")

    # internal DRAM: streamed activations + collective bounce buffers
    h_dram = [nc.dram_tensor(f"h{l}", [NPATCH, nct, 128, nact], D)
              for l in range(2)]
    cc_in = [nc.dram_tensor(f"cc_in{l}", [128, nstat], f32) for l in range(3)]
    cc_out = [nc.dram_tensor(f"cc_out{l}", [128, nstat], f32,
                             addr_space="Shared") for l in range(3)]

    patches = [(y, x) for y in range(5) for x in range(5)]

    with tile.TileContext(nc) as tc:
        import contextlib
        with contextlib.ExitStack() as ctx:
            const = ctx.enter_context(tc.tile_pool(name="const", bufs=1))
            statsp = ctx.enter_context(tc.tile_pool(name="stats", bufs=2))
            coefp = ctx.enter_context(tc.tile_pool(name="coef", bufs=6))
            psp = ctx.enter_context(
                tc.tile_pool(name="ps", bufs=8, space="PSUM"))

            # ---------------- constants ----------------
            cw_sb = const.tile([128, NL, nct, c], D)
            nc.sync.dma_start(
                out=cw_sb[:],
                in_=cw_in[:].rearrange("l (ct p) o -> p l ct o", p=128))
            gam_sb = const.tile([128, NL, nct], f32)
            nc.sync.dma_start(
                out=gam_sb[:],
                in_=gam_in[:].rearrange("l (ct p) -> p l ct", p=128))
            bet_sb = const.tile([128, NL, nct], f32)
            nc.sync.dma_start(
                out=bet_sb[:],
                in_=bet_in[:].rearrange("l (ct p) -> p l ct", p=128))
            eps_sb = const.tile([128, 1], f32)
            nc.vector.memset(eps_sb[:], EPS)
            ones_mm = const.tile([1, 128], D)
            nc.vector.memset(ones_mm[:], 1.0)
            ctx_sb = const.tile([128, nct, NPATCH, bl], D)

            stats_t = [statsp.tile([128, nct, NPATCH, 2], f32, tag="st",
                                   name=f"stats{i}") for i in range(3)]
            coef_a = [coefp.tile([128, nct, NPATCH], f32, tag="cf",
                                 name=f"coefa{i}") for i in range(3)]
            coef_d = [coefp.tile([128, nct, NPATCH], f32, tag="cf",
                                 name=f"coefd{i}") for i in range(3)]

            def emit_allreduce(l):
                flat = stats_t[l][:].rearrange("p a b c -> p (a b c)")
                nc.gpsimd.dma_start(out=cc_in[l][:], in_=flat)
                nc.gpsimd.collective_compute(
                    "AllReduce", _ALU.add,
                    replica_groups=[list(range(ncores))],
                    ins=[cc_in[l][:].opt()],
                    outs=[cc_out[l][:].opt()])
                nc.gpsimd.dma_start(out=flat, in_=cc_out[l][:])

            def emit_coefs(l):
                st = stats_t[l]
                m_t = coefp.tile([128, nct, NPATCH], f32, tag="cf")
                v_t = coefp.tile([128, nct, NPATCH], f32, tag="cf")
                a_t, d_t = coef_a[l], coef_d[l]
                gbc = gam_sb[:, l, :, None].to_broadcast((128, nct, NPATCH))
                bbc = bet_sb[:, l, :, None].to_broadcast((128, nct, NPATCH))
                inv_n = 1.0 / float(ntot)
                nc.vector.tensor_scalar_mul(m_t[:], st[:, :, :, 0], inv_n)
                nc.vector.tensor_tensor(v_t[:], m_t[:], m_t[:], _ALU.mult)
                nc.vector.scalar_tensor_tensor(
                    out=v_t[:], in0=st[:, :, :, 1], scalar=inv_n, in1=v_t[:],
                    op0=_ALU.mult, op1=_ALU.subtract)
                nc.scalar.activation(out=v_t[:], in_=v_t[:], func=_AF.Sqrt,
                                     bias=eps_sb[:], scale=1.0)
                nc.vector.reciprocal(out=v_t[:], in_=v_t[:])
                nc.vector.tensor_tensor(a_t[:], v_t[:], gbc, _ALU.mult)
                nc.vector.tensor_tensor(d_t[:], m_t[:], a_t[:], _ALU.mult)
                nc.vector.tensor_tensor(d_t[:], bbc, d_t[:], _ALU.subtract)

            # ---------------- load x + layer-0 stats ----------------
            with tc.tile_pool(name="xp", bufs=1) as xp, \
                 tc.tile_pool(name="l0scratch", bufs=1) as l0s:
                x_sb = xp.tile([128, nct, bl, HW * HW], f32)
                nc.sync.dma_start(
                    out=x_sb[:],
                    in_=x_in[:].rearrange("(ct p) b x -> p ct b x", p=128))

                # per-pixel sums over batch (patches overlap -> share work)
                sqx = l0s.tile([128, nct, bl, HW * HW], mybir.dt.bfloat16)
                nc.scalar.activation(out=sqx[:], in_=x_sb[:], func=_AF.Square)
                spix = l0s.tile([128, nct, HW * HW], f32)
                sqpix = l0s.tile([128, nct, HW * HW], f32)
                nc.vector.tensor_reduce(
                    out=spix[:],
                    in_=x_sb[:].rearrange("p c b x -> p c x b"),
                    axis=mybir.AxisListType.X, op=_ALU.add)
                nc.vector.tensor_reduce(
                    out=sqpix[:],
                    in_=sqx[:].rearrange("p c b x -> p c x b"),
                    axis=mybir.AxisListType.X, op=_ALU.add)
                for p, (y, x0) in enumerate(patches):
                    sw = spix[:].rearrange("p c (h w) -> p c h w", w=HW)[
                        :, :, y:y + 3, x0:x0 + 3]
                    qw = sqpix[:].rearrange("p c (h w) -> p c h w", w=HW)[
                        :, :, y:y + 3, x0:x0 + 3]
                    nc.vector.tensor_reduce(
                        out=stats_t[0][:, :, p, 0:1], in_=sw,
                        axis=mybir.AxisListType.XY, op=_ALU.add)
                    nc.vector.tensor_reduce(
                        out=stats_t[0][:, :, p, 1:2], in_=qw,
                        axis=mybir.AxisListType.XY, op=_ALU.add)
                emit_allreduce(0)
                emit_coefs(0)

                # ---------------- conv layers ----------------
                with tc.tile_pool(name="rhs", bufs=3) as rhsp, \
                     tc.tile_pool(name="raw", bufs=3) as rawp, \
                     tc.tile_pool(name="stg", bufs=3) as stgp, \
                     tc.tile_pool(name="sqb", bufs=2) as sqbp:

                    def conv_layer(l):
                        a_t, d_t = coef_a[l], coef_d[l]
                        for p, (y, x0) in enumerate(patches):
                            # --- normalized+relu rhs tile ---
                            rhs = rhsp.tile([128, nct, nact], D, tag="rhs")
                            for ct in range(nct):
                                if l == 0:
                                    xin = x_sb[:, ct].rearrange(
                                        "p b (h w) -> p b h w", w=HW)[
                                        :, :, y:y + 3, x0:x0 + 3]
                                    rout = rhs[:, ct].rearrange(
                                        "p (b h w) -> p b h w", b=bl, h=3)
                                else:
                                    xin = raw[:, ct]
                                    rout = rhs[:, ct]
                                nc.scalar.activation(
                                    out=rout, in_=xin, func=_AF.Relu,
                                    scale=a_t[:, ct, p:p + 1],
                                    bias=d_t[:, ct, p:p + 1])
                            if l < 2:
                                stage = stgp.tile([128, nct, nact], D,
                                                  tag="stg")
                            # --- matmuls + epilogue per output tile ---
                            for ot in range(nct):
                                ps_t = psp.tile([128, 512], f32, tag="ps")
                                pout = ps_t[:, :nact]
                                for ct in range(nct):
                                    nc.tensor.matmul(
                                        pout,
                                        cw_sb[:, l, ct,
                                              ot * 128:(ot + 1) * 128],
                                        rhs[:, ct],
                                        start=(ct == 0), stop=(ct == nct - 1))
                                if l < 2:
                                    # copy->stage (+sum), squares (+sumsq)
                                    nc.vector.tensor_scalar(
                                        out=stage[:, ot], in0=pout,
                                        scalar1=1.0, scalar2=0.0,
                                        op0=_ALU.mult, op1=_ALU.add,
                                        accum_out=stats_t[l + 1][:, ot, p,
                                                                 0:1])
                                    sqbin = sqbp.tile([128, nact], D,
                                                      tag="sqb")
                                    nc.vector.scalar_tensor_tensor(
                                        out=sqbin[:], in0=stage[:, ot],
                                        scalar=1.0, in1=stage[:, ot],
                                        op0=_ALU.mult, op1=_ALU.mult,
                                        accum_out=stats_t[l + 1][:, ot, p,
                                                                 1:2])
                                else:
                                    # avg-pool (sum over 3x3) into ctx
                                    with nc.allow_low_precision(
                                            reason="pool-sum cast to mm dtype"):
                                        nc.vector.tensor_reduce(
                                            out=ctx_sb[:, ot, p, :],
                                            in_=pout.rearrange(
                                                "p (b x) -> p b x", x=KPIX),
                                            axis=mybir.AxisListType.X,
                                            op=_ALU.add)
                            if l < 2:
                                nc.sync.dma_start(
                                    out=h_dram[l][p].rearrange(
                                        "c q n -> q c n"),
                                    in_=stage[:])

                    # layer 0 (reads x_sb), then AR+coefs, layer 1, AR, layer 2
                    raw = None
                    conv_layer(0)
                    emit_allreduce(1)
                    emit_coefs(1)
                    for l in (1, 2):
                        for_p = []  # noqa - clarity only
                        a_t, d_t = coef_a[l], coef_d[l]
                        for p, (y, x0) in enumerate(patches):
                            raw = rawp.tile([128, nct, nact], D, tag="raw")
                            nc.sync.dma_start(
                                out=raw[:],
                                in_=h_dram[l - 1][p].rearrange(
                                    "c q n -> q c n"))
                            rhs = rhsp.tile([128, nct, nact], D, tag="rhs")
                            for ct in range(nct):
                                nc.scalar.activation(
                                    out=rhs[:, ct], in_=raw[:, ct],
                                    func=_AF.Relu,
                                    scale=a_t[:, ct, p:p + 1],
                                    bias=d_t[:, ct, p:p + 1])
                            if l < 2:
                                stage = stgp.tile([128, nct, nact], D,
                                                  tag="stg")
                            for ot in range(nct):
                                ps_t = psp.tile([128, 512], f32, tag="ps")
                                pout = ps_t[:, :nact]
                                for ct in range(nct):
                                    nc.tensor.matmul(
                                        pout,
                                        cw_sb[:, l, ct,
                                              ot * 128:(ot + 1) * 128],
                                        rhs[:, ct],
                                        start=(ct == 0), stop=(ct == nct - 1))
                                if l < 2:
                                    nc.vector.tensor_scalar(
                                        out=stage[:, ot], in0=pout,
                                        scalar1=1.0, scalar2=0.0,
                                        op0=_ALU.mult, op1=_ALU.add,
                                        accum_out=stats_t[l + 1][:, ot, p,
                                                                 0:1])
                                    sqbin = sqbp.tile([128, nact], D,
                                                      tag="sqb")
                                    nc.vector.scalar_tensor_tensor(
                                        out=sqbin[:], in0=stage[:, ot],
                                        scalar=1.0, in1=stage[:, ot],
                                        op0=_ALU.mult, op1=_ALU.mult,
                                        accum_out=stats_t[l + 1][:, ot, p,
                                                                 1:2])
                                else:
                                    with nc.allow_low_precision(
                                            reason="pool-sum cast to mm dtype"):
                                        nc.vector.tensor_reduce(
                                            out=ctx_sb[:, ot, p, :],
                                            in_=pout.rearrange(
                                                "p (b x) -> p b x", x=KPIX),
                                            axis=mybir.AxisListType.X,
                                            op=_ALU.add)
                            if l < 2:
                                nc.sync.dma_start(
                                    out=h_dram[l][p].rearrange(
                                        "c q n -> q c n"),
                                    in_=stage[:])
                        if l == 1:
                            emit_allreduce(2)
                            emit_coefs(2)

            # ---------------- prediction heads ----------------
            with tc.tile_pool(name="lwp", bufs=2) as lwp, \
                 tc.tile_pool(name="lbp", bufs=2) as lbp, \
                 tc.tile_pool(name="pkp", bufs=2) as pkp, \
                 tc.tile_pool(name="hsp", bufs=4) as hsp:
                for h in range(NHEADS):
                    d = h // 3
                    lw_sb = lwp.tile([128, nct, c], D, tag="lw")
                    nc.sync.dma_start(
                        out=lw_sb[:],
                        in_=lw_in[h].rearrange("(ct p) o -> p ct o", p=128))
                    lb_sb = lbp.tile([1, c], D, tag="lb")
                    nc.sync.dma_start(out=lb_sb[:], in_=lb_in[h:h + 1, :])
                    packed = pkp.tile([128, nct, 10, bl], D, tag="pk")
                    if d == 0:
                        nc.vector.tensor_copy(out=packed[:],
                                              in_=ctx_sb[:, :, 0:10, :])
                    elif d == 1:
                        nc.vector.tensor_copy(out=packed[:],
                                              in_=ctx_sb[:, :, 15:25, :])
                    else:
                        e0 = 0 if d == 2 else 3
                        src = ctx_sb[:].rearrange(
                            "p c (g f) b -> p c g f b", g=5)[
                            :, :, :, e0:e0 + 2, :]
                        nc.vector.tensor_copy(
                            out=packed[:].rearrange(
                                "p c (g f) b -> p c g f b", g=5),
                            in_=src)
                    for (p0, npat) in mt_groups:
                        M = npat * bl
                        hstage = hsp.tile([128, c], f32, tag="hs")
                        for nh in range(n_out_chunks):
                            o0 = nh * 512
                            olen = min(512, c - o0)
                            ps_t = psp.tile([128, 512], f32, tag="ps")
                            pout = ps_t[:M, :olen]
                            nc.tensor.matmul(
                                pout, ones_mm[0:1, 0:M],
                                lb_sb[0:1, o0:o0 + olen],
                                start=True, stop=False)
                            for ct in range(nct):
                                nc.tensor.matmul(
                                    pout, packed[:, ct, p0:p0 + npat, :],
                                    lw_sb[:, ct, o0:o0 + olen],
                                    start=False, stop=(ct == nct - 1))
                            nc.scalar.copy(out=hstage[:M, o0:o0 + olen],
                                           in_=pout)
                        nc.sync.dma_start(
                            out=preds_out[h, p0:p0 + npat].rearrange(
                                "q b o -> (q b) o"),
                            in_=hstage[:M])

    nc.compile()
    return nc


# ---------------- host side ----------------
_built = {}


def _get_nc(key, **kw):
    if key not in _built:
        _built[key] = build_nc(**kw)
    return _built[key]


def _host_prep(x, bn_gamma, bn_beta, conv_w, conv_b, lin_w, lin_b,
               ncores, dt_str):
    _, np_dt = _dt_pair(dt_str)
    B, C = x.shape[0], x.shape[1]
    bl = B // ncores
    x = np.ascontiguousarray(np.asarray(x, dtype=np.float32))
    conv_w = np.asarray(conv_w, dtype=np.float32)
    conv_b = np.asarray(conv_b, dtype=np.float32)
    lin_w = np.asarray(lin_w, dtype=np.float32)
    lin_b = np.asarray(lin_b, dtype=np.float32)

    cw_t = np.ascontiguousarray(conv_w.transpose(0, 2, 1)).astype(np_dt)
    lw_eff = np.zeros((NHEADS, C, C), dtype=np.float32)
    lb_eff = np.zeros((NHEADS, C), dtype=np.float32)
    for d in range(4):
        for s in range(3):
            h = d * 3 + s
            lw_eff[h] = lin_w[d, s].T / 9.0
            lb_eff[h] = lin_b[d, s] + lin_w[d, s] @ conv_b[2]
    lw_t = lw_eff.astype(np_dt)
    lb_t = lb_eff.astype(np_dt)
    gam_t = np.ascontiguousarray(bn_gamma.astype(np.float32))
    bet_t = np.ascontiguousarray(bn_beta.astype(np.float32))

    xr = x.reshape(B, C, HW * HW)
    in_maps = []
    for cid in range(ncores):
        x_t = np.ascontiguousarray(
            xr[cid * bl:(cid + 1) * bl].transpose(1, 0, 2))
        in_maps.append(dict(x_t=x_t, cw_t=cw_t, lw_t=lw_t, lb_t=lb_t,
                            gam_t=gam_t, bet_t=bet_t))
    return in_maps, bl


def kernel(x, bn_gamma, bn_beta, conv_w, conv_b, lin_w, lin_b):
    global LAST_RESULT
    B, C = int(x.shape[0]), int(x.shape[1])
    ncores = NCORES
    bl = B // ncores
    nc = _get_nc((ncores, bl, C, DTYPE), ncores=ncores, bl=bl, c=C,
                 dt_str=DTYPE)
    in_maps, bl = _host_prep(x, bn_gamma, bn_beta, conv_w, conv_b,
                             lin_w, lin_b, ncores, DTYPE)
    res = bass_utils.run_bass_kernel_spmd(
        nc, in_maps, core_ids=list(range(ncores)), trace=TRACE)
    LAST_RESULT = res
    jmap = _pred_index_map()
    out = np.empty((120, B, C), dtype=np.float32)
    for cid in range(ncores):
        ph = res.results[cid]["preds_t"]  # [12, 10, bl, C]
        for h in range(NHEADS):
            out[jmap[h], cid * bl:(cid + 1) * bl, :] = ph[h]
    return out
